# revision 8
# baseline (speedup 1.0000x reference)
"""Trainium2 Bass kernel for nn_Model_22960895164724 (v2, pipelined).

Model: 5 iterations of a Conway-flavored conv block on [4,1,256,256]:
  h = [x, xp, xp>0.5, prob_step(xp), binary_step(xp>0.5)]  (5 ch)
  y1 = relu(conv5x5_wrap(h, 5->256));  y2 = relu(conv3x3_wrap(y1, 256->256))
  y3 = relu(conv1x1(y2, 256->256));    xp' = sigmoid(conv3x3_wrap(y3, 256->1))

Sharding: 8 cores = 4 images x 2 H-halves with a shrinking halo margin
(25,20,15,10,5 rows), no inter-core communication.

v2 scheduling: the DVE stencil for iteration k+1 is emitted inside iteration
k's strip loop (chunk A after strip 8, chunk B after the last strip) so it
overlaps PE conv work. Load DMAs ride the SP HWDGE queue; store-side DMAs
and late-epoch loads ride the gpsimd SWDGE queue; y1/y3 wrap copies run on
the Act engine so the DVE queue carries only stencil + Zt work. conv4's
tap alignment + reduction is 9 gpsimd accumulate-DMAs into a rows-on-
partitions tile followed by one per-strip sigmoid, so PE never waits on it.
F32R rounding happens only at matmul-operand producers (weights, X1 im2col);
the threshold/sigmoid paths stay exact fp32.
"""
import numpy as np

import concourse.bass as bass
import concourse.tile as tile
from concourse import bacc, mybir
from concourse.bass_utils import run_bass_kernel_spmd

F32 = mybir.dt.float32
F32R = mybir.dt.float32r
AF = mybir.ActivationFunctionType
OP = mybir.AluOpType

# margins m_k: xp_k is valid on slab rows [25-m_k, 153+m_k); slab has 178 rows.
MARG = [25, 20, 15, 10, 5, 0]
SLAB = 178          # local rows: global row g = (r0 - 25 + l) mod 256
WP = 260            # padded width: col jp <-> j = (jp-2) mod 256
R_STRIP = 16
N_IT = 5

_CACHE = {}


def _strips(lo, hi, step):
    out = []
    t = lo
    while t < hi:
        out.append((t, min(t + step, hi)))
        t += step
    return out


def _ab_ranges(lo, hi):
    """Split slab row range [lo,hi) into (tile_idx, tile_lo, tile_hi) pieces
    across xpA (rows 0..127) / xpB (rows 128..SLAB)."""
    pieces = []
    if lo < 128:
        pieces.append((0, lo, min(hi, 128)))
    if hi > 128:
        pieces.append((1, max(lo, 128) - 128, hi - 128))
    return pieces


def build_nc():
    nc = bacc.Bacc("TRN2", target_bir_lowering=False, debug=False, num_devices=8)

    x_slab = nc.dram_tensor("x_slab", [SLAB, WP], F32, kind="ExternalInput")
    w1T = nc.dram_tensor("w1T", [125, 2, 128], F32, kind="ExternalInput")
    b1 = nc.dram_tensor("b1", [128, 2], F32, kind="ExternalInput")
    w2T = nc.dram_tensor("w2T", [128, 2, 2, 9, 128], F32, kind="ExternalInput")
    b2 = nc.dram_tensor("b2", [128, 2], F32, kind="ExternalInput")
    w3T = nc.dram_tensor("w3T", [128, 2, 2, 128], F32, kind="ExternalInput")
    b3 = nc.dram_tensor("b3", [128, 2], F32, kind="ExternalInput")
    w4T = nc.dram_tensor("w4T", [128, 2, 9], F32, kind="ExternalInput")
    b4 = nc.dram_tensor("b4", [128, 1], F32, kind="ExternalInput")
    out = nc.dram_tensor("out", [128, 256], F32, kind="ExternalOutput")

    with tile.TileContext(nc) as tc:
        with (
            tc.tile_pool(name="cons", bufs=1) as cons,
            tc.tile_pool(name="xp_pool", bufs=2) as xp_pool,
            tc.tile_pool(name="sten", bufs=1) as sten,
            tc.tile_pool(name="stage", bufs=1) as stage,
            tc.tile_pool(name="x1p", bufs=1) as x1p,
            tc.tile_pool(name="y1p", bufs=1) as y1p,
            tc.tile_pool(name="y2p", bufs=2) as y2p,
            tc.tile_pool(name="y3p", bufs=2) as y3p,
            tc.tile_pool(name="zp", bufs=1) as zp,
            tc.tile_pool(name="o4", bufs=2) as o4,
            tc.tile_pool(name="ps", bufs=4, space="PSUM") as ps,
            tc.tile_pool(name="psz", bufs=4, space="PSUM") as psz,
        ):
            # ---- constants ----
            w1s = cons.tile([125, 2, 128], F32R, tag="w1s")
            w2s = cons.tile([128, 2, 2, 9, 128], F32R, tag="w2s")
            w3s = cons.tile([128, 2, 2, 128], F32R, tag="w3s")
            w4s = cons.tile([128, 2, 9], F32R, tag="w4s")
            b1s = cons.tile([128, 2], F32, tag="b1s")
            b2s = cons.tile([128, 2], F32, tag="b2s")
            b3s = cons.tile([128, 2], F32, tag="b3s")
            b4s = cons.tile([128, 1], F32, tag="b4s")
            nc.sync.dma_start(w1s[:], w1T[:].bitcast(F32R))
            nc.sync.dma_start(w2s[:], w2T[:].bitcast(F32R))
            nc.sync.dma_start(w3s[:], w3T[:].bitcast(F32R))
            nc.sync.dma_start(w4s[:], w4T[:].bitcast(F32R))
            nc.sync.dma_start(b1s[:], b1[:])
            nc.sync.dma_start(b2s[:], b2[:])
            nc.sync.dma_start(b3s[:], b3[:])
            nc.sync.dma_start(b4s[:], b4[:])

            # ---- x slab (constant across iterations), rows-part, 2 tiles ----
            xsA = cons.tile([128, WP], F32, tag="xsA")
            xsB = cons.tile([SLAB - 128, WP], F32, tag="xsB")
            nc.sync.dma_start(xsA[:], x_slab[0:128, :])
            nc.sync.dma_start(xsB[:], x_slab[128:SLAB, :])

            def slab_dma(dst, dst_r0, src_pair, lo, hi, chan=None, eng=None):
                """dst[(chan,) dst_r0 : dst_r0+(hi-lo), :] = slab rows [lo,hi)."""
                eng = eng or nc.sync
                for ti, a, b_ in _ab_ranges(lo, hi):
                    src = src_pair[ti]
                    off = dst_r0 + (a + 128 * ti - lo)
                    d = (dst[off : off + (b_ - a), :] if chan is None
                         else dst[chan : chan + 1, off : off + (b_ - a), :])
                    eng.dma_start(d, src[a:b_, :])

            def emit_stencil(xp_pair, k_it, which, h_fields_out):
                """Stencil chunk `which` (0/1/2) for iteration k_it: bin/pred/
                predbin on slab rows [lo, lo+n). Loads ride gpsimd (stores
                epoch); compute on DVE. Chunks: 48/80/rest rows so consumers
                unblock early."""
                m1 = MARG[k_it + 1]
                h_lo, h_hi = 25 - m1 - 4, 153 + m1 + 4
                lo, n = [(h_lo, 48), (h_lo + 48, 76),
                         (h_lo + 124, h_hi - h_lo - 124)][which]
                if n <= 0:
                    return
                ctr = sten.tile([128, WP], F32, tag="ctr")
                up = sten.tile([128, WP], F32, tag="up")
                dn = sten.tile([128, WP], F32, tag="dn")
                slab_dma(ctr, 0, xp_pair, lo, lo + n, eng=nc.gpsimd)
                slab_dma(up, 0, xp_pair, lo + 1, lo + n + 1, eng=nc.gpsimd)
                slab_dma(dn, 0, xp_pair, lo - 1, lo + n - 1, eng=nc.gpsimd)

                hf = sten.tile([128, 3, WP], F32, tag=f"hf{which}")  # bin,pred,predbin
                binc = sten.tile([128, 3, WP], F32, tag="binc")
                V = nc.vector
                cN, cW = n, WP - 2  # compute center cols [1, WP-1)
                for i, srcT in enumerate((ctr, up, dn)):
                    V.tensor_scalar(binc[:cN, i, :], srcT[:cN, :], 0.5, None, OP.is_gt)
                s = sten.tile([128, WP], F32, tag="s")
                t0_ = sten.tile([128, WP], F32, tag="t0_")
                # sum of 8 neighbors of bin
                V.tensor_add(s[:cN, 1:1 + cW], binc[:cN, 1, 1:1 + cW], binc[:cN, 2, 1:1 + cW])
                for i, co in ((0, 0), (0, 2), (1, 0), (1, 2), (2, 0), (2, 2)):
                    V.tensor_add(s[:cN, 1:1 + cW], s[:cN, 1:1 + cW], binc[:cN, i, co:co + cW])
                # predbin = (s==3) + bin*(s==2)
                V.tensor_scalar(t0_[:cN, 1:1 + cW], s[:cN, 1:1 + cW], 2.0, None, OP.is_equal)
                V.tensor_mul(t0_[:cN, 1:1 + cW], t0_[:cN, 1:1 + cW], binc[:cN, 0, 1:1 + cW])
                V.tensor_scalar(s[:cN, 1:1 + cW], s[:cN, 1:1 + cW], 3.0, None, OP.is_equal)
                V.tensor_add(hf[:cN, 2, 1:1 + cW], s[:cN, 1:1 + cW], t0_[:cN, 1:1 + cW])
                V.tensor_copy(hf[:cN, 0, 1:1 + cW], binc[:cN, 0, 1:1 + cW])
                # prob DP over the 8 neighbors
                c0t = sten.tile([128, WP], F32, tag="c0t")
                c1t = sten.tile([128, WP], F32, tag="c1t")
                c2t = sten.tile([128, WP], F32, tag="c2t")
                c3t = sten.tile([128, WP], F32, tag="c3t")
                omq = sten.tile([128, WP], F32, tag="omq")
                V.memset(c0t[:cN, :], 1.0)
                V.memset(c1t[:cN, :], 0.0)
                V.memset(c2t[:cN, :], 0.0)
                V.memset(c3t[:cN, :], 0.0)
                for i, co in ((0, 0), (0, 2), (1, 0), (1, 1), (1, 2), (2, 0), (2, 1), (2, 2)):
                    q = (ctr, up, dn)[i]
                    qs = q[:cN, co:co + cW]
                    for hi_t, lo_t in ((c3t, c2t), (c2t, c1t), (c1t, c0t)):
                        V.tensor_sub(t0_[:cN, 1:1 + cW], lo_t[:cN, 1:1 + cW], hi_t[:cN, 1:1 + cW])
                        V.tensor_mul(t0_[:cN, 1:1 + cW], t0_[:cN, 1:1 + cW], qs)
                        V.tensor_add(hi_t[:cN, 1:1 + cW], hi_t[:cN, 1:1 + cW], t0_[:cN, 1:1 + cW])
                    V.tensor_scalar(omq[:cN, 1:1 + cW], qs, -1.0, 1.0, OP.mult, OP.add)
                    V.tensor_mul(c0t[:cN, 1:1 + cW], c0t[:cN, 1:1 + cW], omq[:cN, 1:1 + cW])
                # pred = c3 + c2 * xp
                V.tensor_mul(t0_[:cN, 1:1 + cW], c2t[:cN, 1:1 + cW], ctr[:cN, 1:1 + cW])
                V.tensor_add(hf[:cN, 1, 1:1 + cW], c3t[:cN, 1:1 + cW], t0_[:cN, 1:1 + cW])
                # wrap cols
                V.tensor_copy(hf[:cN, :, 0:1], hf[:cN, :, 256:257])
                V.tensor_copy(hf[:cN, :, WP - 1:WP], hf[:cN, :, 3:4])
                h_fields_out.append((hf, lo, n))

            def hfield_dma(h_fields, dst, chan, fi, lo, hi):
                """dst[chan] rows <- stencil field fi rows [lo,hi) (slab coords)."""
                for hf, base, n in h_fields:
                    a = max(lo, base)
                    b_ = min(hi, base + n)
                    if a < b_:
                        nc.sync.dma_start(
                            dst[chan : chan + 1, (a - lo) : (b_ - lo), :],
                            hf[a - base : b_ - base, fi, :],
                        )

            def emit_strip(t0, t1, xp_pair, nx_pair, h_fields, wrap_nx,
                           pending=None, post_pending=None):
                xpA, xpB = xp_pair
                nxA, nxB = nx_pair
                R = t1 - t0
                # - h5 staging [5, R+8, WP]
                h5 = stage.tile([5, R_STRIP + 9, WP], F32, tag="h5")
                slab_dma(h5, 0, (xsA, xsB), t0 - 4, t1 + 4, chan=0)
                slab_dma(h5, 0, (xpA, xpB), t0 - 4, t1 + 4, chan=1)
                for fi in range(3):
                    hfield_dma(h_fields, h5, 2 + fi, fi, t0 - 4, t1 + 4)
                # - im2col X1 [125, R+4, WP]: one contiguous F32R (rounding) DMA per tap
                X1 = x1p.tile([125, R_STRIP + 5, WP], F32R, tag="X1")
                h5f = h5.bitcast(F32R).rearrange("c r j -> c (r j)")
                X1f = X1.rearrange("(c t) r j -> c t (r j)", t=25)
                nflat = (R + 4) * WP
                for di in range(5):
                    for dj in range(5):
                        nc.sync.dma_start(
                            X1f[:, di * 5 + dj, 0:nflat],
                            h5f[:, di * WP + dj : di * WP + dj + nflat],
                        )
                # - conv1 -> y1 (center cols 2..258)
                y1 = y1p.tile([128, 2, R_STRIP + 4, WP], F32R, tag="y1")
                for rr in range(0, R + 4, 2):
                    for oc in range(2):
                        psum = ps.tile([128, 2, 256], F32, tag="ps")
                        nc.tensor.matmul(
                            psum[:], w1s[:, oc, :], X1[:, rr:rr + 2, 0:256],
                            start=True, stop=True,
                        )
                        nc.scalar.activation(
                            y1[:, oc, rr:rr + 2, 2:258], psum[:],
                            AF.Relu, bias=b1s[:, oc:oc + 1],
                        )
                for oc in range(2):
                    nc.scalar.activation(y1[:, oc, 0:R + 4, 0:2], y1[:, oc, 0:R + 4, 256:258], AF.Copy)
                    nc.scalar.activation(y1[:, oc, 0:R + 4, 258:260], y1[:, oc, 0:R + 4, 2:4], AF.Copy)

                # - conv2/conv3/conv4-z over y2 subtiles of 4 rows
                Zt = zp.tile([9, R_STRIP + 3, 258], F32, tag="Zt")
                for u0 in range(0, R + 2, 4):
                    u1 = min(u0 + 4, R + 2)
                    y2 = y2p.tile([128, 2, 4, 256], F32R, tag="y2")
                    for uu in range(u0, u1, 2):
                        un = min(2, u1 - uu)
                        for oc in range(2):
                            psum = ps.tile([128, 2, 256], F32, tag="ps")
                            kk = 0
                            for ic in range(2):
                                for tap in range(9):
                                    di, dj = tap // 3, tap % 3
                                    nc.tensor.matmul(
                                        psum[:, 0:un, :],
                                        w2s[:, ic, oc, tap, :],
                                        y1[:, ic, uu + di : uu + di + un, dj + 1 : dj + 257],
                                        start=(kk == 0), stop=(kk == 17),
                                    )
                                    kk += 1
                            nc.scalar.activation(
                                y2[:, oc, uu - u0 : uu - u0 + un, :], psum[:, 0:un, :],
                                AF.Relu, bias=b2s[:, oc:oc + 1],
                            )
                    # conv3 -> y3 subtile
                    y3 = y3p.tile([128, 2, 4, 258], F32R, tag="y3")
                    for uu in range(u0, u1, 2):
                        un = min(2, u1 - uu)
                        for oc in range(2):
                            psum = ps.tile([128, 2, 256], F32, tag="ps")
                            for ic in range(2):
                                nc.tensor.matmul(
                                    psum[:, 0:un, :],
                                    w3s[:, ic, oc, :],
                                    y2[:, ic, uu - u0 : uu - u0 + un, :],
                                    start=(ic == 0), stop=(ic == 1),
                                )
                            nc.scalar.activation(
                                y3[:, oc, uu - u0 : uu - u0 + un, 1:257], psum[:, 0:un, :],
                                AF.Relu, bias=b3s[:, oc:oc + 1],
                            )
                    for oc in range(2):
                        nc.scalar.activation(y3[:, oc, 0:u1 - u0, 0:1], y3[:, oc, 0:u1 - u0, 256:257], AF.Copy)
                        nc.scalar.activation(y3[:, oc, 0:u1 - u0, 257:258], y3[:, oc, 0:u1 - u0, 1:2], AF.Copy)
                    # conv4 z: per row, z[9, 258] = sum_ic w4T[ic]^T @ y3row
                    for uu in range(u0, u1):
                        pz = psz.tile([9, 258], F32, tag="pz")
                        for ic in range(2):
                            nc.tensor.matmul(
                                pz[:], w4s[:, ic, :], y3[:, ic, uu - u0, :],
                                start=(ic == 0), stop=(ic == 1),
                            )
                        nc.vector.tensor_copy(Zt[:, uu, :], pz[:])
                    # previous strip's deferred output block, gated on this
                    # strip's y2 so the scheduler keeps its Act fence late
                    if u0 == 0:
                        if pending is not None:
                            pending(dep_y2=y2)
                        if post_pending is not None:
                            post_pending()
                # - tap reduction: 4 independent accumulate-DMA chains into
                #   column slices of zsp, then a 3-add combine on gpsimd
                zsp = o4.tile([R_STRIP, 4, 256], F32, tag="zsp")
                for p, taps in enumerate(([0, 4, 8], [1, 5], [2, 6], [3, 7])):
                    for j, tap in enumerate(taps):
                        di, dj = tap // 3, tap % 3
                        nc.gpsimd.dma_start(
                            zsp[0:R, p, :],
                            Zt[tap : tap + 1, di : di + R, dj : dj + 256],
                            accum_op=(OP.bypass if j == 0 else OP.add),
                        )
                zsum = o4.tile([R_STRIP, 256], F32, tag="zsum")
                nc.gpsimd.tensor_add(zsum[0:R, :], zsp[0:R, 0, :], zsp[0:R, 1, :])
                nc.gpsimd.tensor_add(zsum[0:R, :], zsum[0:R, :], zsp[0:R, 2, :])
                nc.gpsimd.tensor_add(zsum[0:R, :], zsum[0:R, :], zsp[0:R, 3, :])
                # - deferred output block: sigmoid + scatter + wrap cols
                def backend_out(dep_y2=None):
                    ob2 = o4.tile([R_STRIP, 256], F32, tag="ob2")
                    if dep_y2 is not None:
                        b4d = o4.tile([128, 1], F32, tag="b4d")
                        nc.gpsimd.tensor_scalar(b4d[:], dep_y2[0:128, 0, 0, 0:1], 0.0, None, OP.mult)
                        nc.gpsimd.tensor_add(b4d[:], b4d[:], b4s[:, 0:1])
                        bias_ap = b4d[0:R, 0:1]
                    else:
                        bias_ap = b4s[0:R, 0:1]
                    nc.scalar.activation(ob2[0:R, :], zsum[0:R, :], AF.Sigmoid, bias=bias_ap)
                    for ti, a, b_ in _ab_ranges(t0, t1):
                        dst = (nxA, nxB)[ti]
                        nc.gpsimd.dma_start(
                            dst[a:b_, 2:258],
                            ob2[(a + 128 * ti - t0) : (b_ + 128 * ti - t0), :],
                        )
                        if wrap_nx:
                            nc.gpsimd.tensor_copy(dst[a:b_, 0:2], dst[a:b_, 256:258])
                            nc.gpsimd.tensor_copy(dst[a:b_, 258:260], dst[a:b_, 2:4])
                return backend_out

            # ---- main loop: strips with pipelined next-iteration stencil ----
            xp_tiles = (xsA, xsB)  # xp_0 = x
            h_fields = []
            for w in range(3):
                emit_stencil(xp_tiles, 0, w, h_fields)
            pending = None
            for k in range(N_IT):
                m1 = MARG[k + 1]
                out_lo, out_hi = 25 - m1, 153 + m1
                nxA = xp_pool.tile([128, WP], F32, tag="nxA")
                nxB = xp_pool.tile([SLAB - 128, WP], F32, tag="nxB")
                h_fields_next = []
                strips = _strips(out_lo, out_hi, R_STRIP)
                for i, (t0, t1) in enumerate(strips):
                    # chunk c0 after backend(3) (inside strip 4); c1 after
                    # backend(8) (inside strip 9)
                    post = None
                    if k + 1 < N_IT and pending is not None:
                        if i == 4:
                            post = (lambda nk=k + 1, hf=h_fields_next:
                                    emit_stencil((nxA, nxB), nk, 0, hf))
                        elif i == 8:
                            post = (lambda nk=k + 1, hf=h_fields_next:
                                    emit_stencil((nxA, nxB), nk, 1, hf))
                    pending = emit_strip(t0, t1, xp_tiles, (nxA, nxB), h_fields,
                                         wrap_nx=(k + 1 < N_IT),
                                         pending=pending, post_pending=post)
                # end of iteration: flush last backend, then remaining chunks
                if k + 1 < N_IT:
                    pending()
                    pending = None
                    emit_stencil((nxA, nxB), k + 1, 2, h_fields_next)
                h_fields = h_fields_next
                xp_tiles = (nxA, nxB)
            pending()

            # ---- output: xp_5 rows [25,153), cols 2..258 ----
            fA, fB = xp_tiles
            nc.sync.dma_start(out[0:103, :], fA[25:128, 2:258])
            nc.sync.dma_start(out[103:128, :], fB[0:25, 2:258])

    nc.finalize()
    return nc


def _host_inputs(x, w1, b1, w2, b2, w3, b3, w4, b4):
    """Build the 8 per-core input dicts (host-side slicing/transposes)."""
    B, _, H, W = x.shape
    xx = x[:, 0]  # [4,256,256]

    def pad_wrap_cols(a):  # [rows,256] -> [rows,260]
        return np.concatenate([a[:, -2:], a, a[:, :2]], axis=1)

    # lhsT[(c,di,dj), oc, o] = w1[oc*128+o, c, di, dj]
    w1T = np.ascontiguousarray(
        w1.reshape(2, 128, 5, 5, 5).transpose(2, 3, 4, 0, 1).reshape(125, 2, 128)
    )
    w2T = np.ascontiguousarray(
        w2.reshape(2, 128, 2, 128, 3, 3).transpose(3, 2, 0, 4, 5, 1)
        .reshape(128, 2, 2, 9, 128)
    )  # [k(ic ch), ic, oc, tap, o]
    w3T = np.ascontiguousarray(
        w3.reshape(2, 128, 2, 128, 1, 1)[..., 0, 0].transpose(3, 2, 0, 1)
        .reshape(128, 2, 2, 128)
    )
    w4T = np.ascontiguousarray(
        w4.reshape(1, 2, 128, 3, 3).transpose(2, 1, 0, 3, 4).reshape(128, 2, 9)
    )
    shared = {
        "w1T": w1T.astype(np.float32),
        "b1": np.ascontiguousarray(b1.reshape(2, 128).T).astype(np.float32),
        "w2T": w2T.astype(np.float32),
        "b2": np.ascontiguousarray(b2.reshape(2, 128).T).astype(np.float32),
        "w3T": w3T.astype(np.float32),
        "b3": np.ascontiguousarray(b3.reshape(2, 128).T).astype(np.float32),
        "w4T": w4T.astype(np.float32),
        "b4": np.full((128, 1), float(np.asarray(b4).reshape(-1)[0]), np.float32),
    }
    in_maps = []
    for c in range(8):
        b_, half = c // 2, c % 2
        r0 = 128 * half
        rows = (r0 - 25 + np.arange(SLAB)) % 256
        slab = pad_wrap_cols(xx[b_][rows]).astype(np.float32)
        in_maps.append({**shared, "x_slab": np.ascontiguousarray(slab)})
    return in_maps


def kernel(x, w1, b1, w2, b2, w3, b3, w4, b4, n_it):
    assert int(n_it) == N_IT
    x = np.asarray(x, np.float32)
    if "nc" not in _CACHE:
        _CACHE["nc"] = build_nc()
    nc = _CACHE["nc"]
    in_maps = _host_inputs(
        x, np.asarray(w1, np.float32), np.asarray(b1, np.float32),
        np.asarray(w2, np.float32), np.asarray(b2, np.float32),
        np.asarray(w3, np.float32), np.asarray(b3, np.float32),
        np.asarray(w4, np.float32), np.asarray(b4, np.float32),
    )
    res = run_bass_kernel_spmd(nc, in_maps, core_ids=list(range(8)))
    out = np.zeros((4, 1, 256, 256), np.float32)
    for c in range(8):
        b_, half = c // 2, c % 2
        out[b_, 0, 128 * half : 128 * half + 128, :] = res.results[c]["out"]
    return out
